# revision 12
# baseline (speedup 1.0000x reference)
"""Multi-head attention layer (B=4, L=S=2048, D=512, H=8) on 8 trn2 NeuronCores.

Sharding: 32 (batch, head) units -> each core owns 1 batch x 4 heads.
Core c: batch b = c // 2, heads g*4 .. g*4+3 with g = c % 2.
Returns (out [B,L,D], attn [B,H,L,S]) like the reference.

Per-core device program (all matmuls in float32r = full-rate fp32):
  - QKV projections from host-transposed inputs (d-major), biases via
    per-partition ACT bias (Q,K) / ones-row matmul (V).
  - Per head: "LS" pass computes scores[l,s], exp via ScalarE with
    accum_out giving the softmax row-sum Z for free, normalize on
    VectorE, DMA the attn tile out (contiguous 1MB writes).
    "SL" pass recomputes scores transposed [s,l], exp, and accumulates
    U^T = V^T E^T on the PE.  Softmax division is folded into the
    output projection as a per-partition scale (rows = l there).
  - out_partial = sum_h (U_h/Z_h) @ Wo_h^T ; host adds the two
    half-batch partials + bo.
"""

import functools
from contextlib import ExitStack

import numpy as np

import concourse.bass as bass
import concourse.bacc as bacc
import concourse.tile as tile
from concourse import mybir
from concourse.bass_utils import run_bass_kernel_spmd

B, L, S, D, H, DH = 4, 2048, 2048, 512, 8, 64
NH = 4            # heads per core
NCORES = 8
SCALE = 1.0 / 8.0  # 1/sqrt(DH)
F32 = mybir.dt.float32
F32R = mybir.dt.float32r
AF = mybir.ActivationFunctionType
ALU = mybir.AluOpType

LCH = L // 128   # 16 l-chunks
SCH = S // 128   # 16 s-chunks
DCH = D // 128   # 4 d-chunks


def build_nc():
    nc = bacc.Bacc(None)

    qT = nc.dram_tensor("qT", [D, L], F32R, kind="ExternalInput")
    kT = nc.dram_tensor("kT", [D, S], F32R, kind="ExternalInput")
    vT = nc.dram_tensor("vT", [D, S], F32R, kind="ExternalInput")
    wqt = nc.dram_tensor("wqt", [D, NH * DH], F32R, kind="ExternalInput")
    wkt = nc.dram_tensor("wkt", [D, NH * DH], F32R, kind="ExternalInput")
    wvt = nc.dram_tensor("wvt", [D, NH * DH], F32R, kind="ExternalInput")
    wot = nc.dram_tensor("wot", [NH * DH, D], F32R, kind="ExternalInput")
    bqc = nc.dram_tensor("bqc", [128, 2], F32, kind="ExternalInput")
    bkc = nc.dram_tensor("bkc", [128, 2], F32, kind="ExternalInput")
    bvr = nc.dram_tensor("bvr", [1, NH * DH], F32R, kind="ExternalInput")
    onesd = nc.dram_tensor("ones", [1, S], F32R, kind="ExternalInput")

    attn_out = nc.dram_tensor("attn_out", [NH, L, S], F32, kind="ExternalOutput")
    pout = nc.dram_tensor("pout", [L, D], F32, kind="ExternalOutput")

    with tile.TileContext(nc) as tc, ExitStack() as ctx:
        # ---------- persistent tiles ----------
        pers = ctx.enter_context(tc.tile_pool(name="pers", bufs=1))
        QT = pers.tile([128, 2, L], F32R)      # [p, j, l]: head 2j + p//64, dh p%64
        KT = pers.tile([128, 2, S], F32R)
        V = pers.tile([128, SCH, NH * DH], F32R)   # [s%128, s//128, h*64+dh]
        UT = pers.tile([128, 2, L], F32R)      # like QT: U^T per head (unnormalized)
        Zrecip = pers.tile([128, NH, LCH], F32)   # 1/Z per (head, lchunk) col-layout
        wo_s = pers.tile([128, 2, D], F32R)    # wot rows (local dh) split in 2
        bq_s = pers.tile([128, 2], F32)
        bk_s = pers.tile([128, 2], F32)
        bv_s = pers.tile([1, NH * DH], F32R)
        ones_row = pers.tile([1, S], F32R)
        nc.sync.dma_start(out=ones_row, in_=onesd[:, :])

        nc.sync.dma_start(out=wo_s[:, 0, :], in_=wot[0:128, :])
        nc.sync.dma_start(out=wo_s[:, 1, :], in_=wot[128:256, :])
        nc.sync.dma_start(out=bq_s, in_=bqc[:, :])
        nc.sync.dma_start(out=bk_s, in_=bkc[:, :])
        nc.sync.dma_start(out=bv_s, in_=bvr[:, :])

        # ---------- load inputs + projections ----------
        with tc.tile_pool(name="inp", bufs=1) as inp, \
             tc.tile_pool(name="wns", bufs=1) as wns, \
             tc.tile_pool(name="pproj", bufs=4, space="PSUM") as pproj:
            qT_a = inp.tile([128, DCH, L], F32R, name="qTa")
            kT_a = inp.tile([128, DCH, S], F32R, name="kTa")
            vT_a = inp.tile([128, DCH, S], F32R, name="vTa")
            wq_a = wns.tile([128, DCH, NH * DH], F32R, name="wqa")
            wk_a = wns.tile([128, DCH, NH * DH], F32R, name="wka")
            wv_a = wns.tile([128, DCH, NH * DH], F32R, name="wva")
            for (dst, srcd) in ((qT_a, qT), (kT_a, kT), (vT_a, vT),
                                (wq_a, wqt), (wk_a, wkt), (wv_a, wvt)):
                nc.sync.dma_start(out=dst, in_=srcd.rearrange("(c p) n -> p c n", p=128))
            qT_s = [qT_a[:, c, :] for c in range(DCH)]
            kT_s = [kT_a[:, c, :] for c in range(DCH)]
            vT_s = [vT_a[:, c, :] for c in range(DCH)]
            wq_s = [wq_a[:, c, :] for c in range(DCH)]
            wk_s = [wk_a[:, c, :] for c in range(DCH)]
            wv_s = [wv_a[:, c, :] for c in range(DCH)]

            # "touch" matmuls: the self-loading f32r Matmult can carry only a
            # single sync wait in walrus codegen, so advance the PE's vector
            # clock over every DMA lane / engine sem a real matmul would
            # otherwise have to wait on (one tiny matmul = one wait each).
            with tc.tile_pool(name="touch", bufs=1, space="PSUM") as tpp:
                tps = tpp.tile([1, 16], F32)
                touches = [qT_a[:, 0, :], kT_a[:, 0, :], vT_a[:, 0, :],
                           wq_a[:, 0, :], wk_a[:, 0, :], wv_a[:, 0, :],
                           wo_s[:, 0, :], bv_s, ones_row]
                for i, t in enumerate(touches):
                    tf = t[0:1, 0:1].bitcast(F32)
                    nc.tensor.matmul(tps[0:1, i:i + 1], tf, tf,
                                     start=True, stop=True)

            # Q^T, K^T: out [dh-pair rows, l]  = W^T.T @ x^T
            for (xs, ws, dstT, bias) in ((qT_s, wq_s, QT, bq_s), (kT_s, wk_s, KT, bk_s)):
                for j in range(2):
                    for n in range(L // 512):
                        ps = pproj.tile([128, 512], F32, tag="pp")
                        for c in range(DCH):
                            nc.tensor.matmul(
                                ps,
                                ws[c][:, j * 128:(j + 1) * 128],
                                xs[c][:, n * 512:(n + 1) * 512],
                                start=(c == 0), stop=(c == DCH - 1),
                            )
                        nc.scalar.activation(
                            dstT[:, j, n * 512:(n + 1) * 512], ps,
                            AF.Identity, bias=bias[:, j:j + 1],
                        )
            # V natural: out [s rows, 4*64] = v^T.T @ W^T  (+ ones-row * bias-row)
            for sc in range(SCH):
                ps = pproj.tile([128, NH * DH], F32, tag="pp")
                for c in range(DCH):
                    nc.tensor.matmul(
                        ps,
                        vT_s[c][:, sc * 128:(sc + 1) * 128],
                        wv_s[c],
                        start=(c == 0), stop=False,
                    )
                nc.tensor.matmul(
                    ps,
                    ones_row[0:1, sc * 128:(sc + 1) * 128].bitcast(F32),
                    bv_s[0:1, :].bitcast(F32),
                    start=False, stop=True,
                )
                nc.vector.tensor_copy(V[:, sc, :], ps)

            # touch V so later U-matmuls see the DVE copies' sem as observed
            with tc.tile_pool(name="touch2", bufs=1, space="PSUM") as tpp2:
                tps2 = tpp2.tile([1, 1], F32)
                vf = V[0:1, SCH - 1, 0:1].bitcast(F32)
                nc.tensor.matmul(tps2[0:1, 0:1], vf, vf, start=True, stop=True)

        # ---------- attention per local head ----------
        for u in range(NH):
            j, poff = u // 2, (u % 2) * 64
            QTh = QT[poff:poff + 64, j, :]
            KTh = KT[poff:poff + 64, j, :]

            # --- LS pass: attn rows out ---
            with tc.tile_pool(name=f"lsp{u}", bufs=2, space="PSUM") as lsp, \
                 tc.tile_pool(name=f"lse{u}", bufs=8) as lse:
                for lc in range(LCH):
                    ps = lsp.tile([128, S], F32, tag="s")
                    for n in range(S // 512):
                        nc.tensor.matmul(
                            ps[:, n * 512:(n + 1) * 512],
                            QTh[:, lc * 128:(lc + 1) * 128],
                            KTh[:, n * 512:(n + 1) * 512],
                            start=True, stop=True,
                        )
                    e = lse.tile([128, S], F32, tag="e")
                    zc = Zrecip[:, u, lc:lc + 1]
                    nc.scalar.activation(e, ps, AF.Exp, scale=SCALE, accum_out=zc)
                    nc.vector.reciprocal(zc, zc)
                    nc.vector.tensor_scalar_mul(e, e, zc)
                    nc.sync.dma_start(
                        out=attn_out[u, lc * 128:(lc + 1) * 128, :], in_=e,
                    )

            # --- SL pass: U^T accumulation ---
            with tc.tile_pool(name=f"slp{u}", bufs=2, space="PSUM") as slp, \
                 tc.tile_pool(name=f"slu{u}", bufs=1, space="PSUM") as slu, \
                 tc.tile_pool(name=f"sle{u}", bufs=3) as sle:
                Ups = slu.tile([64, L], F32, tag="u")
                Vh = V[:, :, u * 64:(u + 1) * 64]
                for sc in range(SCH):
                    for hf in range(2):
                        ps = slp.tile([128, L // 2], F32, tag="s2")
                        for n in range(2):
                            nc.tensor.matmul(
                                ps[:, n * 512:(n + 1) * 512],
                                KTh[:, sc * 128:(sc + 1) * 128],
                                QTh[:, hf * 1024 + n * 512:hf * 1024 + (n + 1) * 512],
                                start=True, stop=True,
                            )
                        e2 = sle.tile([128, L // 2], F32R, tag="e2")
                        nc.scalar.activation(e2, ps, AF.Exp, scale=SCALE)
                        for n in range(2):
                            nc.tensor.matmul(
                                Ups[:, hf * 1024 + n * 512:hf * 1024 + (n + 1) * 512],
                                Vh[:, sc, :],
                                e2[:, n * 512:(n + 1) * 512],
                                start=(sc == 0), stop=(sc == SCH - 1),
                                skip_group_check=True,
                            )
                nc.vector.tensor_copy(UT[poff:poff + 64, j, :], Ups)

        # ---------- output projection, normalized per head ----------
        with tc.tile_pool(name="opp", bufs=2, space="PSUM") as opp, \
             tc.tile_pool(name="opo", bufs=3) as opo:
            for lc in range(LCH):
                acc = opo.tile([128, D], F32, tag="acc")
                for u in range(NH):
                    j, poff = u // 2, (u % 2) * 64
                    po = opp.tile([128, D], F32, tag="po")
                    nc.tensor.matmul(
                        po,
                        UT[poff:poff + 64, j, lc * 128:(lc + 1) * 128],
                        wo_s[poff:poff + 64, j, :],
                        start=True, stop=True,
                    )
                    zc = Zrecip[:, u, lc:lc + 1]
                    if u == 0:
                        nc.vector.tensor_scalar_mul(acc, po, zc)
                    else:
                        nc.vector.scalar_tensor_tensor(
                            acc, po, zc, acc, ALU.mult, ALU.add,
                        )
                nc.sync.dma_start(out=pout[lc * 128:(lc + 1) * 128, :], in_=acc)

    nc.finalize()
    return nc


@functools.lru_cache(maxsize=1)
def _nc_cached():
    return build_nc()


def kernel(queries, keys, values, Wq, bq, Wk, bk, Wv, bv, Wo, bo):
    queries = np.ascontiguousarray(queries, np.float32)
    keys = np.ascontiguousarray(keys, np.float32)
    values = np.ascontiguousarray(values, np.float32)

    in_maps = []
    for c in range(NCORES):
        b, g = c // 2, c % 2
        h0 = g * NH
        sl = slice(h0 * DH, (h0 + NH) * DH)
        in_maps.append({
            "qT": np.ascontiguousarray(queries[b].T),
            "kT": np.ascontiguousarray(keys[b].T),
            "vT": np.ascontiguousarray(values[b].T),
            "wqt": np.ascontiguousarray(np.asarray(Wq, np.float32)[sl, :].T),
            "wkt": np.ascontiguousarray(np.asarray(Wk, np.float32)[sl, :].T),
            "wvt": np.ascontiguousarray(np.asarray(Wv, np.float32)[sl, :].T),
            "wot": np.ascontiguousarray(np.asarray(Wo, np.float32)[:, sl].T),
            "bqc": np.ascontiguousarray(np.asarray(bq, np.float32)[sl].reshape(2, 128).T),
            "bkc": np.ascontiguousarray(np.asarray(bk, np.float32)[sl].reshape(2, 128).T),
            "bvr": np.ascontiguousarray(np.asarray(bv, np.float32)[sl].reshape(1, -1)),
            "ones": np.ones((1, S), np.float32),
        })

    nc = _nc_cached()
    res = run_bass_kernel_spmd(nc, in_maps, core_ids=list(range(NCORES))).results

    attn = np.empty((B, H, L, S), np.float32)
    out = np.empty((B, L, D), np.float32)
    bo = np.asarray(bo, np.float32)
    for b in range(B):
        for g in range(2):
            attn[b, g * NH:(g + 1) * NH] = res[2 * b + g]["attn_out"]
        out[b] = res[2 * b]["pout"] + res[2 * b + 1]["pout"] + bo
    return out, attn


if __name__ == "__main__":
    nc = build_nc()
    print("build ok:", len(nc.m.functions[0].allocations), "allocations")
